# revision 19
# baseline (speedup 1.0000x reference)
"""GrwSmoothingLoss on 8 Trainium2 NeuronCores.

Math: for each batch b, with Gram matrix G_b = Z_b @ Z_b^T (8x8),
  logits[b,p] = -0.5 * ||diff2(Z_b[perm_p])||^2 = -0.5 * <C_p, G_b>,
where C_p = P_p^T C2 P_p with C2 = D2^T D2 the fixed 8x8 second-difference
Gram. Since P_p is a permutation, C_p[u,v] = C2[inv_p[u], inv_p[v]] — a pure
gather of a constant 8x8 matrix, so the whole coefficient table (1000 perm
columns + the 0.5*alpha*C1 smoothness column) is precomputed on host from
perm_index. G is symmetric, so only 48 ordered pair-products cover all 64
entries: block A = (u<4, all v) and block B = (u>=4, v>=4); the table folds
the weight of each missing (u>=4, v<4) pair into its mirror slot in A. All
table entries are small integers: exact in fp16.

Device work per core (32 batches): pair products (fp16, k split 4 ways so
all 128 partitions work; A-block + B-block split across DVE and gpsimd), a
k'-reduce, and a fold matmul with a q4 selector -> gT [48, 32] fp16; four
fp16 matmuls put logits as [128=(chunk,b), 250] in PSUM so the exp uses all
128 partitions; exp with accum_out (logits are in [-48, 0] so no max shift
is needed in fp32); a small matmul folds chunk sums + identity logit + V
column to per-batch [s_b, 0.5*X0_b + alpha*V_b]; ln(s_b) lands next to the
linear term in F[32, 2], which is DMA'd out. Host sums the 8 cores'
per-batch partials and divides by B. A manually emitted InstLoadActFuncSet
for the set containing BOTH exp and ln keeps the 1.3us activation-table
load off the critical path (one early load instead of a reload before ln).

Sharding: data-parallel over B (32 batches/core); coefficient table
replicated.
"""

import numpy as np

import concourse.bacc as bacc
import concourse.bass as bass
import concourse.mybir as mybir
import concourse.tile as tile
from concourse.bass_utils import run_bass_kernel_spmd

B, T, K = 256, 8, 128
NUM_PERMS = 1000
ALPHA = 0.5
N_CORES = 8
B_LOC = B // N_CORES
PCHUNK = NUM_PERMS // 4
NSLOT = 48  # 32 A-block + 16 B-block ordered pair slots
F32 = mybir.dt.float32
F16 = mybir.dt.float16

# act_func_sets index of "natural_log_exp_and_others" (contains exp, ln,
# copy, identity) in both placeholder and pwp act_info.json
ACT_SET_LN_EXP = 6

_cache = {}
# calibration aid: timeline_sim's executor can't ln/exp garbage data; Copy
# costs the same on the Act engine so loop-overhead sims swap them out
_ACT_STUB = False


def _consts():
    D2 = (np.eye(T, k=2) - 2 * np.eye(T, k=1) + np.eye(T))[: T - 2]
    C2 = (D2.T @ D2).astype(np.float64)
    D1 = (np.eye(T, k=1) - np.eye(T))[: T - 1]
    C1 = (D1.T @ D1).astype(np.float64)
    # pb32[:, 0:32] = fold4 (sums partition groups 32c+b over c)
    pb32 = np.zeros((128, 32), np.float32)
    for c in range(4):
        pb32[32 * c : 32 * c + 32, :] = np.eye(B_LOC, dtype=np.float32)
    return C2, C1, pb32


def _fold_sym(Cfull):
    """[.., 8, 8] symmetric coeff -> [.., 48] slots (A: u<4 all v; B: u,v>=4)
    with mirror weights for the uncovered (u>=4, v<4) pairs folded into A."""
    W_A = Cfull[..., 0:4, :].copy()  # [.., 4, 8]
    W_A[..., :, 4:8] += np.swapaxes(Cfull, -1, -2)[..., 0:4, 4:8]
    W_B = Cfull[..., 4:8, 4:8]  # [.., 4, 4]
    return np.concatenate(
        [W_A.reshape(*Cfull.shape[:-2], 32), W_B.reshape(*Cfull.shape[:-2], 16)],
        axis=-1,
    )


def _emit_act_preload(nc):
    nc.scalar.add_instruction(
        mybir.InstLoadActFuncSet(
            name=nc.get_next_instruction_name(),
            ins=[],
            outs=[],
            act_func_set_id=ACT_SET_LN_EXP,
        )
    )


def _kernel_body(tc, out_part, zb_d, c16_d, pb32_d, preload_act=True):
    nc = tc.nc
    with (
        tc.tile_pool(name="sb", bufs=1) as sb,
        tc.tile_pool(name="ps", bufs=1, space="PSUM") as ps,
    ):
        if preload_act:
            _emit_act_preload(nc)
        # padded to a full 2KB bank so the matmul outputs stay bank-aligned
        psum_X = ps.tile([128, PCHUNK + 1], F32, padded_shape=[128, 512])
        zb = sb.tile([128, 256 + 32], F16)  # cols 0:256 Z, 256:288 q4
        c16 = sb.tile([NSLOT, NUM_PERMS + 1], F16)
        pb32 = sb.tile([128, 32], F32)
        nc.sync.dma_start(out=zb[:], in_=zb_d[:])
        nc.sync.dma_start(out=c16[:], in_=c16_d[:])
        nc.sync.dma_start(out=pb32[:], in_=pb32_d[:])

        # Gram pair products pp[(b,q), slot, k']; A-block on DVE, B on gpsimd
        pp = sb.tile([128, NSLOT * 32], F16)
        ppv = pp[:].rearrange("p (s k) -> p s k", k=32)
        zv = zb[:, 0:256].rearrange("p (t k) -> p t k", t=8)
        ppA = ppv[:, 0:32, :].rearrange("p (u v) k -> p u v k", v=8)
        nc.vector.tensor_tensor(
            out=ppA,
            in0=zv[:, 0:4, :].unsqueeze(2).broadcast_to([128, 4, 8, 32]),
            in1=zv[:].unsqueeze(1).broadcast_to([128, 4, 8, 32]),
            op=mybir.AluOpType.mult,
        )
        ppB = ppv[:, 32:48, :].rearrange("p (u v) k -> p u v k", v=4)
        nc.vector.tensor_tensor(
            out=ppB,
            in0=zv[:, 4:8, :].unsqueeze(2).broadcast_to([128, 4, 4, 32]),
            in1=zv[:, 4:8, :].unsqueeze(1).broadcast_to([128, 4, 4, 32]),
            op=mybir.AluOpType.mult,
        )
        # k'-reduce as a halving add tree: TensorTensor gets the 2x fp16 DVE
        # rate that TensorReduce lacks (1375 vs 1660 ns modeled)
        gq = sb.tile([128, NSLOT], F16)
        cur = ppv
        width = 32
        while width > 1:
            width //= 2
            if width == 1:
                nxt_t = gq
            else:
                nxt_t = sb.tile([128, NSLOT * width], F16, name=f"tree{width}")
            nxt = nxt_t[:].rearrange("p (s k) -> p s k", k=width)
            nc.vector.tensor_tensor(
                out=nxt,
                in0=cur[:, :, 0:width],
                in1=cur[:, :, width : 2 * width],
                op=mybir.AluOpType.add,
            )
            cur = nxt
        # q-fold + transpose to [slot, b] in one matmul with the q4 selector
        psum_g = ps.tile([NSLOT, B_LOC], F32)
        nc.tensor.matmul(psum_g[:], gq[:], zb[:, 256:288])
        gT = sb.tile([NSLOT, B_LOC], F16)
        nc.vector.tensor_copy(gT[:], psum_g[:])

        # logits (unscaled): X[(c,b), p'] = <G_b, C_{250c+p'}>, V col at
        # chunk 3 col 250
        for c in range(4):
            ncols = PCHUNK + (1 if c == 3 else 0)
            nc.tensor.matmul(
                psum_X[32 * c : 32 * c + 32, 0:ncols],
                gT[:],
                c16[:, PCHUNK * c : PCHUNK * c + ncols],
                tile_position=(0, 32 * c),
            )

        # A[:,0] = sum_p' exp(-0.5 X); col 1 holds 0.5*X0 on partitions 0:32
        # and alpha*V on partitions 96:128 so the fold matmul adds them per b
        e = sb.tile([128, PCHUNK], F32)
        A = sb.tile([128, 2], F32)
        nc.vector.memset(A[:, 1:2], 0.0)
        exp_fn = (
            mybir.ActivationFunctionType.Copy
            if _ACT_STUB
            else mybir.ActivationFunctionType.Exp
        )
        nc.scalar.activation(
            e[:], psum_X[:, 0:PCHUNK], exp_fn, scale=-0.5, accum_out=A[:, 0:1],
        )
        nc.scalar.mul(A[0:32, 1:2], psum_X[0:32, 0:1], 0.5)
        nc.scalar.copy(A[96:128, 1:2], psum_X[96:128, PCHUNK : PCHUNK + 1])

        # fold chunks: psum_s[b, :] = [s_b, 0.5*X0_b + alpha*V_b]
        psum_s = ps.tile([B_LOC, 2], F32)
        nc.tensor.matmul(psum_s[:], pb32[:], A[:])

        # F[b] = [ln s_b, 0.5*X0_b + alpha*V_b]; host sums all and adds
        F = sb.tile([B_LOC, 2], F32)
        ln_fn = (
            mybir.ActivationFunctionType.Copy
            if _ACT_STUB
            else mybir.ActivationFunctionType.Ln
        )
        nc.scalar.activation(F[:, 0:1], psum_s[:, 0:1], ln_fn)
        nc.scalar.copy(F[:, 1:2], psum_s[:, 1:2])
        nc.sync.dma_start(out=out_part[:], in_=F[:])


def _build(n_iters=1):
    if ("nc", n_iters) in _cache:
        return _cache[("nc", n_iters)]
    nc = bacc.Bacc(
        "TRN2",
        target_bir_lowering=False,
        debug=False,
        enable_asserts=False,
        num_devices=N_CORES,
    )
    zb_d = nc.dram_tensor("zb", [128, 288], F16, kind="ExternalInput").ap()
    c16_d = nc.dram_tensor(
        "c16", [NSLOT, NUM_PERMS + 1], F16, kind="ExternalInput"
    ).ap()
    pb32_d = nc.dram_tensor("pb32", [128, 32], F32, kind="ExternalInput").ap()
    out_d = nc.dram_tensor("out_part", [B_LOC, 2], F32, kind="ExternalOutput").ap()
    with tile.TileContext(nc) as tc:
        if n_iters == 1:
            _kernel_body(tc, out_d, zb_d, c16_d, pb32_d)
        else:
            # hoist the act-table preload out of the loop so the bench's
            # per-iteration delta matches the single-shot kernel
            _emit_act_preload(nc)
            with tc.For_i(0, n_iters, 1):
                _kernel_body(tc, out_d, zb_d, c16_d, pb32_d, preload_act=False)
    nc.compile()
    _cache[("nc", n_iters)] = nc
    return nc


def _in_maps(Z, perm_index):
    perm = np.asarray(perm_index, dtype=np.int64).reshape(NUM_PERMS, T)
    inv = np.argsort(perm, axis=1)
    C2, C1, pb32 = _consts()
    ctab = _fold_sym(C2[inv[:, :, None], inv[:, None, :]])  # [P, 48]
    c1col = _fold_sym(0.5 * ALPHA * C1)  # [48]
    c16 = np.concatenate([ctab.T, c1col[:, None]], axis=1).astype(np.float16)
    q4 = np.repeat(np.eye(B_LOC, dtype=np.float16), 4, axis=0)
    Zf = np.asarray(Z, dtype=np.float32).reshape(B, T, 4, 32)
    in_maps = []
    for c in range(N_CORES):
        zb4 = (
            np.ascontiguousarray(
                Zf[c * B_LOC : (c + 1) * B_LOC].transpose(0, 2, 1, 3)
            )
            .reshape(128, 256)
            .astype(np.float16)
        )
        in_maps.append(
            {"zb": np.concatenate([zb4, q4], axis=1), "c16": c16, "pb32": pb32}
        )
    return in_maps


def kernel(Z, perm_index, _trace=False):
    nc = _build()
    in_maps = _in_maps(Z, perm_index)
    res = run_bass_kernel_spmd(
        nc, in_maps, core_ids=list(range(N_CORES)), trace=_trace
    )
    total = np.float64(0.0)
    for r in res.results:
        total += np.float64(r["out_part"].astype(np.float64).sum())
    out = np.array(total / B, dtype=np.float32)
    if _trace:
        return out, res
    return out
